# revision 22
# baseline (speedup 1.0000x reference)
"""Trainium2 Bass kernel for nn_Connection_v5 (geodesic-spray-style RHS).

Math (per sample n, D=128, 2D=256):
    x = input_[:, :D], v = input_[:, D:]
    z1 = x @ W1.T + b1            [2D]
    h  = relu(z1), mask = z1 > 0  [2D]
    s  = sigmoid(h @ W2.T + b2)   [D]
    sign_j = -1 if j < 4 else 1
    g  = (s + 0.618) * sign;  jac[i,j] = sign_i s_i(1-s_i) * (W2 (mask*W1))[i,j]
    dv[j] = -1/g_j * sum_i v_i^2 jac[i,j] + 2 v_j / g_j * sum_i v_i jac[j,i]
    out = [v, dv]

Folded form used here (signs/scales pushed into host-precomputed weights):
    nsps = (s-1)*s            (= -s(1-s))
    gr   = 1/(s+0.618)
    [z1 | u] = W1b @ [x | v]  (single bf16 matmul, fused M1+M3)
    mu   = (h>0) * u
    v2   = v*v ; wt = v2*nsps ; p = v*nsps
    at   = wt @ (sign_i*W2) ; am = (h>0) * at
    At   = am @ (W1*sign_j) ; Ct = mu @ (-2*W2.T)
    dv   = gr * (At + p*Ct)

Sharding: pure data-parallel over N=8192 across 8 cores (1024 rows each);
weights replicated.  On-chip layout is feature-major [feat, n]; sample-major
<->feature-major conversion via PE transposes with an identity matrix.
All matmuls bf16 (fp32 PSUM accumulate); the relu mask comes from the
bf16-input z1 (measured ~7e-3 output rel err vs the 2e-2 gate).  All
weights arrive in two packed DMAs on the sync HWDGE queue so compute can
start ~3us in (SWDGE weight loads were costing ~8us of fill).
"""

import os
import numpy as np

D = 128
TWO_D = 256
N_TOTAL = 8192
NCORES = 8
N_CORE = N_TOTAL // NCORES  # 1024
NF = 256                    # samples per pipeline chunk (matmul moving dim)
CONST = 0.618
SIGN = 4
DEPTH = 3                   # backB(c - DEPTH + 1) emission lag

_CACHE = {}

# bwall column layout (bf16): [w1b | w2t | w2sgn | w1sgn | w2t2 | idnb]
_W1B, _W2T, _W2SGN, _W1SGN, _W2T2, _IDNB = (
    0, TWO_D, 2 * TWO_D, 3 * TWO_D, 4 * TWO_D, 5 * TWO_D)
_BWALL = 5 * TWO_D + 128
# fwall column layout (f32): [idn | b1(2) | b2 | c618]
_FWALL = 128 + 4


def _build(n_core=N_CORE):
    """Build + compile the per-core Bass module (cached)."""
    from contextlib import ExitStack

    import concourse.bacc as bacc
    import concourse.mybir as mybir
    import concourse.tile as tile

    f32 = mybir.dt.float32
    bf16 = mybir.dt.bfloat16
    Act = mybir.ActivationFunctionType
    Op = mybir.AluOpType

    nchunk = n_core // NF
    nb = NF // 128  # 128-row blocks per chunk

    nc = bacc.Bacc("TRN2", target_bir_lowering=False, debug=False,
                   num_devices=NCORES)

    inp = nc.dram_tensor("inp", [n_core, TWO_D], f32, kind="ExternalInput").ap()
    fwall = nc.dram_tensor("fwall", [128, _FWALL], f32,
                           kind="ExternalInput").ap()
    bwall = nc.dram_tensor("bwall", [128, _BWALL], bf16,
                           kind="ExternalInput").ap()
    out = nc.dram_tensor("out", [n_core, TWO_D], f32, kind="ExternalOutput").ap()

    with tile.TileContext(nc) as tc:
        with ExitStack() as ctx:
            singles = ctx.enter_context(tc.tile_pool(name="singles", bufs=1))
            io = ctx.enter_context(tc.tile_pool(name="io", bufs=4))
            acts = ctx.enter_context(tc.tile_pool(name="acts", bufs=4))
            psum = ctx.enter_context(
                tc.tile_pool(name="psum", bufs=1, space="PSUM"))

            # all constants/weights in two HWDGE DMAs, on-chip by ~2.5us
            sb_f = singles.tile([128, _FWALL], f32, name="sb_f")
            nc.sync.dma_start(out=sb_f, in_=fwall)
            sb_w = singles.tile([128, _BWALL], bf16, name="sb_w")
            nc.sync.dma_start(out=sb_w, in_=bwall)
            sb_id = sb_f[:, 0:128]
            sb_b1 = sb_f[:, 128:130]
            sb_b2 = sb_f[:, 130:131]
            sb_c618 = sb_f[:, 131:132]
            sb_idb = sb_w[:, _IDNB:_IDNB + 128]

            # prime the ACT function tables (Relu/Sigmoid/Identity/Copy) so
            # the ~1.3us ACT_TABLE_LOADs overlap the DMAs instead of blocking
            # the first real activation.
            warm = singles.tile([128, 1], f32, name="warm")
            nc.scalar.activation(out=warm, in_=sb_f[:, 0:1],
                                 func=Act.Relu, bias=sb_b2[:, 0:1], scale=1.0)
            nc.scalar.activation(out=warm, in_=sb_f[:, 0:1],
                                 func=Act.Sigmoid, bias=sb_b2[:, 0:1],
                                 scale=1.0)
            nc.scalar.activation(out=warm, in_=sb_f[:, 0:1],
                                 func=Act.Identity, bias=sb_b2[:, 0:1],
                                 scale=1.0)
            nc.scalar.copy(out=warm, in_=sb_f[:, 0:1])

            inp_v = inp.rearrange("(c b p) d -> c p b d", p=128, b=nb)
            outd_v = out[:, D:TWO_D].rearrange("(c b p) d -> c p b d",
                                               p=128, b=nb)
            outv_v = out[:, 0:D].rearrange("(c b p) d -> c p b d",
                                           p=128, b=nb)

            state = {}

            def front_in(c, off, nf, cnb):
                """DMA in, v passthrough, transposes, copies, fused M1+M3."""
                blk = inp[off:off + nf].rearrange("(b p) d -> p b d", p=128)
                inb = io.tile([128, cnb, TWO_D], f32, tag="inb", name="inb")
                nc.sync.dma_start(out=inb, in_=blk)
                # v passthrough straight from SBUF (input already on-chip)
                ov = out[off:off + nf, 0:D].rearrange("(b p) d -> p b d",
                                                      p=128)
                nc.sync.dma_start(out=ov, in_=inb[:, :, D:TWO_D])

                # sample-major -> feature-major via PE transposes
                ps_tr = psum.tile([128, 2, nf], f32, tag="tp", bufs=2,
                                  name="ps_tr")
                for b in range(cnb):
                    nc.tensor.transpose(ps_tr[:, 0, 128 * b:128 * (b + 1)],
                                        inb[:, b, 0:D], sb_id)
                    nc.tensor.transpose(ps_tr[:, 1, 128 * b:128 * (b + 1)],
                                        inb[:, b, D:TWO_D], sb_id)
                # bf16 [x | v] for the fused M1+M3 moving operand
                xvb = acts.tile([128, 2, nf], bf16, tag="xvb", name="xvb")
                nc.scalar.copy(out=xvb, in_=ps_tr)
                v2 = acts.tile([128, nf], f32, tag="v2", name="v2")
                nc.gpsimd.tensor_tensor(v2, xvb[:, 1, :], xvb[:, 1, :],
                                        Op.mult)

                # fused M1+M3: [z1 | u]^T = W1b @ [x | v]^T  (2*nf moving)
                ps_uz = psum.tile([128, 2, 2 * nf], f32, tag="uz", bufs=1,
                                  name="ps_uz")
                xvf = xvb.rearrange("p a n -> p (a n)")
                for k in range(2):
                    nc.tensor.matmul(ps_uz[:, k, :],
                                     sb_w[:, _W1B + 128 * k:_W1B + 128 * (k + 1)],
                                     xvf, start=True, stop=True)
                state[c] = dict(ps_uz=ps_uz, xvb=xvb, v2=v2)

            def front_main(c, off, nf, cnb):
                """relu, mask*u, M2, sigmoid, gr/nsps."""
                st = state[c]
                ps_uz = st.pop("ps_uz")

                h = acts.tile([128, 2, nf], bf16, tag="h", name="h")
                for k in range(2):
                    nc.scalar.activation(out=h[:, k, :], in_=ps_uz[:, k, 0:nf],
                                         func=Act.Relu,
                                         bias=sb_b1[:, k:k + 1], scale=1.0)
                # mu = (h > 0) * u, u read straight from the M13 PSUM
                mu = acts.tile([128, 2, nf], bf16, tag="mu", name="mu")
                nc.vector.scalar_tensor_tensor(
                    out=mu, in0=h, scalar=0.0,
                    in1=ps_uz[:, :, nf:2 * nf],
                    op0=Op.is_gt, op1=Op.mult)

                # M2: z2 = W2 @ h (accumulate over the two k-chunks)
                ps_z2 = psum.tile([128, nf], f32, tag="z2", bufs=1,
                                  name="ps_z2")
                for k in range(2):
                    nc.tensor.matmul(
                        ps_z2, sb_w[:, _W2T + 128 * k:_W2T + 128 * (k + 1)],
                        h[:, k, :], start=(k == 0), stop=(k == 1))
                s = acts.tile([128, nf], f32, tag="s", name="s")
                nc.scalar.activation(out=s, in_=ps_z2, func=Act.Sigmoid,
                                     bias=sb_b2[:, 0:1], scale=1.0)

                nsps = acts.tile([128, nf], f32, tag="nsps", name="nsps")
                nc.vector.scalar_tensor_tensor(out=nsps, in0=s, scalar=-1.0,
                                               in1=s, op0=Op.add, op1=Op.mult)
                gs = acts.tile([128, nf], f32, tag="gs", name="gs")
                nc.scalar.activation(out=gs, in_=s, func=Act.Identity,
                                     bias=sb_c618[:, 0:1], scale=1.0)
                gr = acts.tile([128, nf], f32, tag="gr", name="gr")
                nc.vector.reciprocal_approx_fast(out=gr, in_=gs)
                st.update(h=h, mu=mu, gr=gr, nsps=nsps)

            def backA(c, off, nf, cnb):
                """wt, M4, am."""
                st = state[c]
                h, v2 = st.pop("h"), st.pop("v2")
                nsps = st["nsps"]

                wt = acts.tile([128, nf], bf16, tag="wt", name="wt")
                nc.gpsimd.tensor_tensor(wt, v2, nsps, Op.mult)

                # M4: at^T, contraction over i with (sign_i*W2)
                ps_a = psum.tile([128, 2, nf], f32, tag="a", bufs=1,
                                 name="ps_a")
                for k in range(2):
                    nc.tensor.matmul(
                        ps_a[:, k, :],
                        sb_w[:, _W2SGN + 128 * k:_W2SGN + 128 * (k + 1)],
                        wt, start=True, stop=True)
                am = acts.tile([128, 2, nf], bf16, tag="am", name="am")
                nc.vector.scalar_tensor_tensor(
                    out=am, in0=h, scalar=0.0, in1=ps_a,
                    op0=Op.is_gt, op1=Op.mult)
                st.update(am=am)

            def backB(c, off, nf, cnb):
                st = state.pop(c)
                gr, mu, am = st["gr"], st["mu"], st["am"]
                xvb, nsps = st["xvb"], st["nsps"]

                # p = v*nsps, needed only for the tail combine
                p = acts.tile([128, nf], f32, tag="p", name="p")
                nc.gpsimd.tensor_tensor(p, xvb[:, 1, :], nsps, Op.mult)

                # M6: Ct = mu @ (-2*W2.T) first (tpc consumes it);
                # M5: At = am @ (W1*sign_j)
                ps_AC = psum.tile([128, 2, nf], f32, tag="AC", bufs=1,
                                  name="ps_AC")
                for k in range(2):
                    nc.tensor.matmul(
                        ps_AC[:, 1, :],
                        sb_w[:, _W2T2 + 128 * k:_W2T2 + 128 * (k + 1)],
                        mu[:, k, :], start=(k == 0), stop=(k == 1))
                for k in range(2):
                    nc.tensor.matmul(
                        ps_AC[:, 0, :],
                        sb_w[:, _W1SGN + 128 * k:_W1SGN + 128 * (k + 1)],
                        am[:, k, :], start=(k == 0), stop=(k == 1))

                # dv = gr * (At + p*Ct)
                tpc = acts.tile([128, nf], f32, tag="tpc", name="tpc")
                nc.vector.tensor_tensor(tpc, p, ps_AC[:, 1, :], Op.mult)
                sm = acts.tile([128, nf], f32, tag="sm", name="sm")
                nc.vector.tensor_tensor(sm, ps_AC[:, 0, :], tpc, Op.add)
                dvT = acts.tile([128, nf], bf16, tag="dvT", name="dvT")
                nc.gpsimd.tensor_tensor(dvT, gr, sm, Op.mult)

                # feature-major -> sample-major (bf16 transpose) and store
                ps_dv = psum.tile([128, nf], bf16, tag="tp", bufs=2,
                                  name="ps_dv")
                for b in range(cnb):
                    nc.tensor.transpose(ps_dv[:, 128 * b:128 * (b + 1)],
                                        dvT[:, 128 * b:128 * (b + 1)], sb_idb)
                ob = io.tile([128, cnb, D], f32, tag="ob", name="ob")
                nc.scalar.copy(out=ob, in_=ps_dv.rearrange(
                    "p (b d) -> p b d", b=cnb))
                od = out[off:off + nf, D:TWO_D].rearrange(
                    "(b p) d -> p b d", p=128)
                nc.sync.dma_start(out=od, in_=ob)

            # chunk plan: big chunks for efficiency, small tail chunks so the
            # final (unoverlapped) drain chain is short
            plan = []
            off = 0
            while n_core - off > NF:
                plan.append((off, NF)); off += NF
            while off < n_core:
                plan.append((off, min(128, n_core - off))); off += 128
            nck = len(plan)

            lag = DEPTH - 1
            for c, (off, nf) in enumerate(plan):
                front_in(c, off, nf, nf // 128)
                if c >= 1:
                    o2, n2 = plan[c - 1]
                    backA(c - 1, o2, n2, n2 // 128)
                front_main(c, off, nf, nf // 128)
                if c >= lag:
                    o2, n2 = plan[c - lag]
                    backB(c - lag, o2, n2, n2 // 128)
            backA(nck - 1, *plan[nck - 1], plan[nck - 1][1] // 128)
            for c in range(max(0, nck - lag), nck):
                backB(c, *plan[c], plan[c][1] // 128)

    nc.compile()
    return nc


def _get_nc(n_core=N_CORE):
    key = ("nc", n_core)
    if key not in _CACHE:
        _CACHE[key] = _build(n_core)
    return _CACHE[key]


def _pack_k(mat):
    """[2D, D] -> [128, 2*128] with the k-chunk partition packing the
    matmul stationary slices expect ([p, (c m)] where row = c*128+p)."""
    return np.ascontiguousarray(
        mat.reshape(2, 128, 128).transpose(1, 0, 2).reshape(128, 256))


def _host_weights(W1, b1, W2, b2):
    import ml_dtypes

    W1 = np.asarray(W1, np.float32)
    b1 = np.asarray(b1, np.float32)
    W2 = np.asarray(W2, np.float32)
    b2 = np.asarray(b2, np.float32)
    bf16 = ml_dtypes.bfloat16
    sign = np.where(np.arange(D) < SIGN, -1.0, 1.0).astype(np.float32)

    bwall = np.zeros((128, _BWALL), np.float32)
    bwall[:, _W1B:_W1B + TWO_D] = W1.T                       # [D, 2D]
    bwall[:, _W2T:_W2T + TWO_D] = _pack_k(W2.T.copy())       # [2D, D] packed
    bwall[:, _W2SGN:_W2SGN + TWO_D] = W2 * sign[:, None]     # [D, 2D]
    bwall[:, _W1SGN:_W1SGN + TWO_D] = _pack_k(W1 * sign[None, :])
    bwall[:, _W2T2:_W2T2 + TWO_D] = _pack_k(-2.0 * W2.T.copy())
    bwall[:, _IDNB:_IDNB + 128] = np.eye(128, dtype=np.float32)

    fwall = np.zeros((128, _FWALL), np.float32)
    fwall[:, 0:128] = np.eye(128, dtype=np.float32)
    fwall[:, 128:130] = b1.reshape(2, 128).T
    fwall[:, 130] = b2
    fwall[:, 131] = CONST

    return {
        "fwall": np.ascontiguousarray(fwall),
        "bwall": np.ascontiguousarray(bwall).astype(bf16),
    }


def _run(inp_np, W1, b1, W2, b2, trace=False):
    from concourse.bass_utils import run_bass_kernel_spmd

    nc = _get_nc(N_CORE)
    wmap = _host_weights(W1, b1, W2, b2)
    in_maps = []
    for c in range(NCORES):
        m = dict(wmap)
        m["inp"] = np.ascontiguousarray(
            inp_np[c * N_CORE:(c + 1) * N_CORE], np.float32)
        in_maps.append(m)
    res = run_bass_kernel_spmd(nc, in_maps, list(range(NCORES)), trace=trace)
    out = np.concatenate([r["out"] for r in res.results], axis=0)
    return out, res


def kernel(t=None, input_=None, W1=None, b1=None, W2=None, b2=None, **kw):
    inp_np = np.ascontiguousarray(np.asarray(input_, np.float32))
    trace = bool(int(os.environ.get("KERNEL_TRACE", "0")))
    out, _ = _run(inp_np, W1, b1, W2, b2, trace=trace)
    return out


def run_traced(inputs):
    """Returns (out, exec_time_ns, trace_path). Used by test.py."""
    inp_np = np.ascontiguousarray(np.asarray(inputs["input_"], np.float32))
    out, res = _run(inp_np, inputs["W1"], inputs["b1"], inputs["W2"],
                    inputs["b2"], trace=True)
    trace_path = None
    if res.instructions_and_trace is not None:
        trace_path = res.instructions_and_trace[1]
    return out, res.exec_time_ns, trace_path


# revision 24
# speedup vs baseline: 1.0588x; 1.0588x over previous
"""Trainium2 Bass kernel for nn_Connection_v5 (geodesic-spray-style RHS).

Math (per sample n, D=128, 2D=256):
    x = input_[:, :D], v = input_[:, D:]
    z1 = x @ W1.T + b1            [2D]
    h  = relu(z1), mask = z1 > 0  [2D]
    s  = sigmoid(h @ W2.T + b2)   [D]
    sign_j = -1 if j < 4 else 1
    g  = (s + 0.618) * sign;  jac[i,j] = sign_i s_i(1-s_i) * (W2 (mask*W1))[i,j]
    dv[j] = -1/g_j * sum_i v_i^2 jac[i,j] + 2 v_j / g_j * sum_i v_i jac[j,i]
    out = [v, dv]

Folded form used here (signs/scales pushed into host-precomputed weights):
    nsps = (s-1)*s            (= -s(1-s))
    gr   = 1/(s+0.618)
    [z1 | u] = W1b @ [x | v]  (single bf16 matmul, fused M1+M3)
    mu   = (h>0) * u
    v2   = v*v ; wt = v2*nsps ; p = v*nsps
    at   = wt @ (sign_i*W2) ; am = (h>0) * at
    At   = am @ (W1*sign_j) ; Ct = mu @ (-2*W2.T)
    dv   = gr * (At + p*Ct)

Sharding: pure data-parallel over N=8192 across 8 cores (1024 rows each);
weights replicated.  On-chip layout is feature-major [feat, n]; sample-major
<->feature-major conversion via PE transposes with an identity matrix.
All matmuls bf16 (fp32 PSUM accumulate); the relu mask comes from the
bf16-input z1 (measured ~7e-3 output rel err vs the 2e-2 gate).  All
weights arrive in two packed DMAs on the sync HWDGE queue so compute can
start ~3us in (SWDGE weight loads were costing ~8us of fill).
"""

import os
import numpy as np

D = 128
TWO_D = 256
N_TOTAL = 8192
NCORES = 8
N_CORE = N_TOTAL // NCORES  # 1024
NF = 256                    # samples per pipeline chunk (matmul moving dim)
CONST = 0.618
SIGN = 4
DEPTH = 3                   # backB(c - DEPTH + 1) emission lag

_CACHE = {}

# bwall column layout (bf16): [w1b | w2t | w2sgn | w1sgn | w2t2 | idnb]
_W1B, _W2T, _W2SGN, _W1SGN, _W2T2, _IDNB = (
    0, TWO_D, 2 * TWO_D, 3 * TWO_D, 4 * TWO_D, 5 * TWO_D)
_BWALL = 5 * TWO_D + 128
# fwall column layout (f32): [idn | b1(2) | b2 | c618]
_FWALL = 128 + 4


def _build(n_core=N_CORE):
    """Build + compile the per-core Bass module (cached)."""
    from contextlib import ExitStack

    import concourse.bacc as bacc
    import concourse.mybir as mybir
    import concourse.tile as tile

    f32 = mybir.dt.float32
    bf16 = mybir.dt.bfloat16
    Act = mybir.ActivationFunctionType
    Op = mybir.AluOpType

    nchunk = n_core // NF
    nb = NF // 128  # 128-row blocks per chunk

    nc = bacc.Bacc("TRN2", target_bir_lowering=False, debug=False,
                   num_devices=NCORES)

    inp = nc.dram_tensor("inp", [n_core, TWO_D], f32, kind="ExternalInput").ap()
    fwall = nc.dram_tensor("fwall", [128, _FWALL], f32,
                           kind="ExternalInput").ap()
    bwall = nc.dram_tensor("bwall", [128, _BWALL], bf16,
                           kind="ExternalInput").ap()
    out = nc.dram_tensor("out", [n_core, TWO_D], f32, kind="ExternalOutput").ap()

    with tile.TileContext(nc) as tc:
        with ExitStack() as ctx:
            singles = ctx.enter_context(tc.tile_pool(name="singles", bufs=1))
            io = ctx.enter_context(tc.tile_pool(name="io", bufs=5))
            acts = ctx.enter_context(tc.tile_pool(name="acts", bufs=5))
            psum = ctx.enter_context(
                tc.tile_pool(name="psum", bufs=1, space="PSUM"))

            # all constants/weights in two HWDGE DMAs, on-chip by ~2.5us
            sb_f = singles.tile([128, _FWALL], f32, name="sb_f")
            nc.sync.dma_start(out=sb_f, in_=fwall)
            sb_w = singles.tile([128, _BWALL], bf16, name="sb_w")
            nc.sync.dma_start(out=sb_w, in_=bwall)
            sb_id = sb_f[:, 0:128]
            sb_b1 = sb_f[:, 128:130]
            sb_b2 = sb_f[:, 130:131]
            sb_c618 = sb_f[:, 131:132]
            sb_idb = sb_w[:, _IDNB:_IDNB + 128]

            # prime the ACT function tables (Relu/Sigmoid/Identity/Copy) so
            # the ~1.3us ACT_TABLE_LOADs overlap the DMAs instead of blocking
            # the first real activation.
            warm = singles.tile([128, 1], f32, name="warm")
            nc.scalar.activation(out=warm, in_=sb_f[:, 0:1],
                                 func=Act.Relu, bias=sb_b2[:, 0:1], scale=1.0)
            nc.scalar.activation(out=warm, in_=sb_f[:, 0:1],
                                 func=Act.Sigmoid, bias=sb_b2[:, 0:1],
                                 scale=1.0)
            nc.scalar.activation(out=warm, in_=sb_f[:, 0:1],
                                 func=Act.Identity, bias=sb_b2[:, 0:1],
                                 scale=1.0)
            nc.scalar.copy(out=warm, in_=sb_f[:, 0:1])

            inp_v = inp.rearrange("(c b p) d -> c p b d", p=128, b=nb)
            outd_v = out[:, D:TWO_D].rearrange("(c b p) d -> c p b d",
                                               p=128, b=nb)
            outv_v = out[:, 0:D].rearrange("(c b p) d -> c p b d",
                                           p=128, b=nb)

            state = {}

            def front_in(c, off, nf, cnb):
                """DMA in, v passthrough, transposes, copies, fused M1+M3."""
                blk = inp[off:off + nf].rearrange("(b p) d -> p b d", p=128)
                inb = io.tile([128, cnb, TWO_D], f32, tag="inb", name="inb")
                nc.sync.dma_start(out=inb, in_=blk)
                # v passthrough straight from SBUF (input already on-chip)
                ov = out[off:off + nf, 0:D].rearrange("(b p) d -> p b d",
                                                      p=128)
                nc.sync.dma_start(out=ov, in_=inb[:, :, D:TWO_D])

                # sample-major -> feature-major via PE transposes
                ps_tr = psum.tile([128, 2, nf], f32, tag="tp", bufs=2,
                                  name="ps_tr")
                for b in range(cnb):
                    nc.tensor.transpose(ps_tr[:, 0, 128 * b:128 * (b + 1)],
                                        inb[:, b, 0:D], sb_id)
                    nc.tensor.transpose(ps_tr[:, 1, 128 * b:128 * (b + 1)],
                                        inb[:, b, D:TWO_D], sb_id)
                # bf16 [x | v] for the fused M1+M3 moving operand
                xvb = acts.tile([128, 2, nf], bf16, tag="xvb", name="xvb")
                nc.scalar.copy(out=xvb, in_=ps_tr)
                v2 = acts.tile([128, nf], f32, tag="v2", name="v2")
                nc.gpsimd.tensor_tensor(v2, xvb[:, 1, :], xvb[:, 1, :],
                                        Op.mult)

                # fused M1+M3: [z1 | u]^T = W1b @ [x | v]^T  (2*nf moving)
                ps_uz = psum.tile([128, 2, 2 * nf], f32, tag="uz", bufs=1,
                                  name="ps_uz")
                xvf = xvb.rearrange("p a n -> p (a n)")
                for k in range(2):
                    nc.tensor.matmul(ps_uz[:, k, :],
                                     sb_w[:, _W1B + 128 * k:_W1B + 128 * (k + 1)],
                                     xvf, start=True, stop=True)
                state[c] = dict(ps_uz=ps_uz, xvb=xvb, v2=v2)

            def front_main(c, off, nf, cnb):
                """relu, mask*u, M2, sigmoid, gr/nsps."""
                st = state[c]
                ps_uz = st.pop("ps_uz")

                h = acts.tile([128, 2, nf], bf16, tag="h", name="h")
                for k in range(2):
                    nc.scalar.activation(out=h[:, k, :], in_=ps_uz[:, k, 0:nf],
                                         func=Act.Relu,
                                         bias=sb_b1[:, k:k + 1], scale=1.0)
                # mu = (h > 0) * u, u read straight from the M13 PSUM
                mu = acts.tile([128, 2, nf], bf16, tag="mu", name="mu")
                nc.vector.scalar_tensor_tensor(
                    out=mu, in0=h, scalar=0.0,
                    in1=ps_uz[:, :, nf:2 * nf],
                    op0=Op.is_gt, op1=Op.mult)

                # M2: z2 = W2 @ h (accumulate over the two k-chunks)
                ps_z2 = psum.tile([128, nf], f32, tag="z2", bufs=1,
                                  name="ps_z2")
                for k in range(2):
                    nc.tensor.matmul(
                        ps_z2, sb_w[:, _W2T + 128 * k:_W2T + 128 * (k + 1)],
                        h[:, k, :], start=(k == 0), stop=(k == 1))
                s = acts.tile([128, nf], f32, tag="s", name="s")
                nc.scalar.activation(out=s, in_=ps_z2, func=Act.Sigmoid,
                                     bias=sb_b2[:, 0:1], scale=1.0)

                nsps = acts.tile([128, nf], f32, tag="nsps", name="nsps")
                nc.vector.scalar_tensor_tensor(out=nsps, in0=s, scalar=-1.0,
                                               in1=s, op0=Op.add, op1=Op.mult)
                gs = acts.tile([128, nf], f32, tag="gs", name="gs")
                nc.scalar.activation(out=gs, in_=s, func=Act.Identity,
                                     bias=sb_c618[:, 0:1], scale=1.0)
                gr = acts.tile([128, nf], f32, tag="gr", name="gr")
                nc.vector.reciprocal_approx_fast(out=gr, in_=gs)
                st.update(h=h, mu=mu, gr=gr, nsps=nsps)

            def backA(c, off, nf, cnb):
                """wt, M4, am."""
                st = state[c]
                h, v2 = st.pop("h"), st.pop("v2")
                nsps = st["nsps"]

                wt = acts.tile([128, nf], bf16, tag="wt", name="wt")
                nc.gpsimd.tensor_tensor(wt, v2, nsps, Op.mult)

                # M4: at^T, contraction over i with (sign_i*W2)
                ps_a = psum.tile([128, 2, nf], f32, tag="a", bufs=1,
                                 name="ps_a")
                for k in range(2):
                    nc.tensor.matmul(
                        ps_a[:, k, :],
                        sb_w[:, _W2SGN + 128 * k:_W2SGN + 128 * (k + 1)],
                        wt, start=True, stop=True)
                am = acts.tile([128, 2, nf], bf16, tag="am", name="am")
                nc.vector.scalar_tensor_tensor(
                    out=am, in0=h, scalar=0.0, in1=ps_a,
                    op0=Op.is_gt, op1=Op.mult)
                st.update(am=am)

            def backB(c, off, nf, cnb):
                st = state.pop(c)
                gr, mu, am = st["gr"], st["mu"], st["am"]
                xvb, nsps = st["xvb"], st["nsps"]

                # p = v*nsps, needed only for the tail combine
                p = acts.tile([128, nf], f32, tag="p", name="p")
                nc.gpsimd.tensor_tensor(p, xvb[:, 1, :], nsps, Op.mult)

                # M6: Ct = mu @ (-2*W2.T) first (tpc consumes it);
                # M5: At = am @ (W1*sign_j)
                ps_AC = psum.tile([128, 2, nf], f32, tag="AC", bufs=1,
                                  name="ps_AC")
                for k in range(2):
                    nc.tensor.matmul(
                        ps_AC[:, 1, :],
                        sb_w[:, _W2T2 + 128 * k:_W2T2 + 128 * (k + 1)],
                        mu[:, k, :], start=(k == 0), stop=(k == 1))
                for k in range(2):
                    nc.tensor.matmul(
                        ps_AC[:, 0, :],
                        sb_w[:, _W1SGN + 128 * k:_W1SGN + 128 * (k + 1)],
                        am[:, k, :], start=(k == 0), stop=(k == 1))

                # dv = gr * (At + p*Ct)
                tpc = acts.tile([128, nf], f32, tag="tpc", name="tpc")
                nc.vector.tensor_tensor(tpc, p, ps_AC[:, 1, :], Op.mult)
                sm = acts.tile([128, nf], f32, tag="sm", name="sm")
                nc.vector.tensor_tensor(sm, ps_AC[:, 0, :], tpc, Op.add)
                dvT = acts.tile([128, nf], bf16, tag="dvT", name="dvT")
                nc.gpsimd.tensor_tensor(dvT, gr, sm, Op.mult)

                # feature-major -> sample-major (bf16 transpose) and store
                ps_dv = psum.tile([128, nf], bf16, tag="tp", bufs=2,
                                  name="ps_dv")
                for b in range(cnb):
                    nc.tensor.transpose(ps_dv[:, 128 * b:128 * (b + 1)],
                                        dvT[:, 128 * b:128 * (b + 1)], sb_idb)
                ob = io.tile([128, cnb, D], f32, tag="ob", name="ob")
                nc.scalar.copy(out=ob, in_=ps_dv.rearrange(
                    "p (b d) -> p b d", b=cnb))
                od = out[off:off + nf, D:TWO_D].rearrange(
                    "(b p) d -> p b d", p=128)
                nc.sync.dma_start(out=od, in_=ob)

            # chunk plan: big chunks for efficiency, small tail chunks so the
            # final (unoverlapped) drain chain is short
            # small chunks at both ends: short fill chain (first chunks ramp
            # the pipe quickly) and short drain chain (last chunk's
            # unoverlapped tail); big chunks in the middle for efficiency
            plan = []
            off = 0
            head = min(2 * 128, max(0, n_core - 2 * NF))
            while off < head:
                plan.append((off, 128)); off += 128
            while n_core - off > NF:
                plan.append((off, NF)); off += NF
            while off < n_core:
                plan.append((off, min(128, n_core - off))); off += 128
            nck = len(plan)

            lag = DEPTH - 1
            for c, (off, nf) in enumerate(plan):
                front_in(c, off, nf, nf // 128)
                if c >= 1:
                    o2, n2 = plan[c - 1]
                    backA(c - 1, o2, n2, n2 // 128)
                front_main(c, off, nf, nf // 128)
                if c >= lag:
                    o2, n2 = plan[c - lag]
                    backB(c - lag, o2, n2, n2 // 128)
            backA(nck - 1, *plan[nck - 1], plan[nck - 1][1] // 128)
            for c in range(max(0, nck - lag), nck):
                backB(c, *plan[c], plan[c][1] // 128)

    nc.compile()
    return nc


def _get_nc(n_core=N_CORE):
    key = ("nc", n_core)
    if key not in _CACHE:
        _CACHE[key] = _build(n_core)
    return _CACHE[key]


def _pack_k(mat):
    """[2D, D] -> [128, 2*128] with the k-chunk partition packing the
    matmul stationary slices expect ([p, (c m)] where row = c*128+p)."""
    return np.ascontiguousarray(
        mat.reshape(2, 128, 128).transpose(1, 0, 2).reshape(128, 256))


def _host_weights(W1, b1, W2, b2):
    import ml_dtypes

    W1 = np.asarray(W1, np.float32)
    b1 = np.asarray(b1, np.float32)
    W2 = np.asarray(W2, np.float32)
    b2 = np.asarray(b2, np.float32)
    bf16 = ml_dtypes.bfloat16
    sign = np.where(np.arange(D) < SIGN, -1.0, 1.0).astype(np.float32)

    bwall = np.zeros((128, _BWALL), np.float32)
    bwall[:, _W1B:_W1B + TWO_D] = W1.T                       # [D, 2D]
    bwall[:, _W2T:_W2T + TWO_D] = _pack_k(W2.T.copy())       # [2D, D] packed
    bwall[:, _W2SGN:_W2SGN + TWO_D] = W2 * sign[:, None]     # [D, 2D]
    bwall[:, _W1SGN:_W1SGN + TWO_D] = _pack_k(W1 * sign[None, :])
    bwall[:, _W2T2:_W2T2 + TWO_D] = _pack_k(-2.0 * W2.T.copy())
    bwall[:, _IDNB:_IDNB + 128] = np.eye(128, dtype=np.float32)

    fwall = np.zeros((128, _FWALL), np.float32)
    fwall[:, 0:128] = np.eye(128, dtype=np.float32)
    fwall[:, 128:130] = b1.reshape(2, 128).T
    fwall[:, 130] = b2
    fwall[:, 131] = CONST

    return {
        "fwall": np.ascontiguousarray(fwall),
        "bwall": np.ascontiguousarray(bwall).astype(bf16),
    }


def _run(inp_np, W1, b1, W2, b2, trace=False):
    from concourse.bass_utils import run_bass_kernel_spmd

    nc = _get_nc(N_CORE)
    wmap = _host_weights(W1, b1, W2, b2)
    in_maps = []
    for c in range(NCORES):
        m = dict(wmap)
        m["inp"] = np.ascontiguousarray(
            inp_np[c * N_CORE:(c + 1) * N_CORE], np.float32)
        in_maps.append(m)
    res = run_bass_kernel_spmd(nc, in_maps, list(range(NCORES)), trace=trace)
    out = np.concatenate([r["out"] for r in res.results], axis=0)
    return out, res


def kernel(t=None, input_=None, W1=None, b1=None, W2=None, b2=None, **kw):
    inp_np = np.ascontiguousarray(np.asarray(input_, np.float32))
    trace = bool(int(os.environ.get("KERNEL_TRACE", "0")))
    out, _ = _run(inp_np, W1, b1, W2, b2, trace=trace)
    return out


def run_traced(inputs):
    """Returns (out, exec_time_ns, trace_path). Used by test.py."""
    inp_np = np.ascontiguousarray(np.asarray(inputs["input_"], np.float32))
    out, res = _run(inp_np, inputs["W1"], inputs["b1"], inputs["W2"],
                    inputs["b2"], trace=True)
    trace_path = None
    if res.instructions_and_trace is not None:
        trace_path = res.instructions_and_trace[1]
    return out, res.exec_time_ns, trace_path
